# revision 8
# baseline (speedup 1.0000x reference)
"""LoRA MLP (gate_up + SiLU*up + down, each with rank-16 LoRA) on 8 TRN2 cores.

Strategy: data-parallel over tokens (16384 = 8 x 2048); weights replicated to
every core, no collectives. All tensors bf16 (PE full rate + FWL fast weight
load + half DMA/SBUF), fp32 PSUM accumulation, fp32 output.

The rank-16 LoRA is folded into the base weights on device:
    W1' = W_gate_up + A_gate_up @ B_gate_up   (PE matmul K=16 + DVE add)
    W2' = W_down    + A_down    @ B_down
so the steady-state loop is a pure dense MLP: clean 8-deep / 22-deep PSUM
accumulation chains with no 16-row LoRA matmuls serializing the PE.

Because the PE instruction queue is strict FIFO, fold work is emitted one
group-pair AHEAD of its consumer and chopped into small units (2 matmuls +
1 DVE add) that are interleaved between the main matmul chunks — by the time
the PE reaches a fold matmul its PSUM slot is long free, so the PE never
stalls on the fold's DVE evacuation chain (which caused HAM re-throttling).

Per core: 2 blocks of 1024 tokens. W2' (44KB/partition) stays SBUF-resident;
W1' groups are folded during block 0 and round-tripped through a DRAM scratch
for block 1 (write and read share one FIFO DMA queue, which orders them).
Activations stay in [feature, token] layout; each LDWEIGHTS feeds 2 matmuls.
DMA spread: weights on sync, x on vector, consts+W2raw on gpsimd, out on
scalar. PSUM: gate 2 + up 2 + shared fold/down-proj 4 banks.
"""

from collections import deque

import numpy as np
import ml_dtypes

import concourse.mybir as mybir
import concourse.tile as tile
from concourse import bacc
from concourse.bass_utils import run_bass_kernel_spmd

TOKENS, D, FF, R = 16384, 1024, 2816, 16
N_CORES = 8
T_CORE = TOKENS // N_CORES  # 2048
BLK = 1024                  # tokens per block (2 blocks/core)
TS = 512                    # psum free-dim tile (1 bank fp32)
DT = D // 128               # 8 d-model tiles
FFT = FF // 128             # 22 ff tiles
NG = 2 * FF // 256          # 22 fold groups of 256 f-cols each
F32 = mybir.dt.float32
BF16 = mybir.dt.bfloat16
SILU = mybir.ActivationFunctionType.Silu
COPY = mybir.ActivationFunctionType.Copy
BF = ml_dtypes.bfloat16

_prog_cache = {}


def _build():
    nc = bacc.Bacc("TRN2", target_bir_lowering=False, debug=False)
    xT = nc.dram_tensor("xT", [D, T_CORE], BF16, kind="ExternalInput").ap()
    w1 = nc.dram_tensor("W_gate_up", [D, 2 * FF], BF16, kind="ExternalInput").ap()
    a1t = nc.dram_tensor("A1T", [R, D], BF16, kind="ExternalInput").ap()
    b1 = nc.dram_tensor("B_gate_up", [R, 2 * FF], BF16, kind="ExternalInput").ap()
    w2 = nc.dram_tensor("W_down", [FF, D], BF16, kind="ExternalInput").ap()
    a2t = nc.dram_tensor("A2T", [R, FF], BF16, kind="ExternalInput").ap()
    b2 = nc.dram_tensor("B_down", [R, D], BF16, kind="ExternalInput").ap()
    out = nc.dram_tensor("out", [T_CORE, D], F32, kind="ExternalOutput").ap()
    # W1' spill space so block 1 re-reads the folded weights instead of refolding
    w1s = nc.dram_tensor("w1s", [NG, 128, DT, 256], BF16, kind="Internal").ap()

    w1r = w1.rearrange("(dt p) f -> p dt f", p=128)   # [128, 8, 5632]
    w2r = w2.rearrange("(ft p) d -> p ft d", p=128)   # [128, 22, 1024]
    xTr = xT.rearrange("(dt p) t -> p dt t", p=128)   # [128, 8, 2048]

    with tile.TileContext(nc) as tc:
        with (
            tc.tile_pool(name="constp", bufs=1) as constp,
            tc.tile_pool(name="w1c", bufs=7) as w1c,      # W1' group tiles, 4KB each
            tc.tile_pool(name="w2p", bufs=1) as w2p,      # W2' resident
            tc.tile_pool(name="w1raw", bufs=3) as w1raw,
            tc.tile_pool(name="w2raw", bufs=2) as w2raw,
            tc.tile_pool(name="b1p", bufs=4) as b1p,
            tc.tile_pool(name="xp", bufs=2) as xp,
            tc.tile_pool(name="hp", bufs=1) as hp,
            tc.tile_pool(name="tmpp", bufs=4) as tmpp,
            tc.tile_pool(name="evp", bufs=2) as evp,
            tc.tile_pool(name="ps", bufs=1, space="PSUM") as ps,
        ):
            a1t_sb = constp.tile([R, D], BF16)
            nc.gpsimd.dma_start(a1t_sb[:], a1t[:])
            a2t_sb = constp.tile([R, FF], BF16)
            nc.gpsimd.dma_start(a2t_sb[:], a2t[:])
            b2_sb = constp.tile([R, D], BF16)
            nc.gpsimd.dma_start(b2_sb[:], b2[:])
            w2sb = w2p.tile([128, FFT, D], BF16)

            pending = deque()  # fold micro-units, drained between main MM chunks

            def drain(n):
                for _ in range(min(n, len(pending))):
                    pending.popleft()()

            def sched_fold_w1(g):
                """Queue fold of W1' cols [g*256,(g+1)*256): 4 units + spill."""
                c0 = g * 256
                raw = w1raw.tile([128, DT, 256], BF16, tag="w1raw")
                nc.sync.dma_start(raw[:], w1r[:, :, c0 : c0 + 256])
                b1c = b1p.tile([R, 256], BF16, tag="b1c")
                nc.sync.dma_start(b1c[:], b1[:, c0 : c0 + 256])
                wt = w1c.tile([128, DT, 256], BF16, tag="w1c")

                def unit(dp):
                    pf = ps.tile([128, 2, 256], F32, tag="pf", bufs=4, name="pf")
                    for k in range(2):
                        dt = 2 * dp + k
                        nc.tensor.matmul(
                            pf[:, k, :],
                            a1t_sb[:, dt * 128 : (dt + 1) * 128],
                            b1c[:],
                            start=True, stop=True,
                        )
                    nc.vector.tensor_add(
                        wt[:, 2 * dp : 2 * dp + 2, :], pf[:],
                        raw[:, 2 * dp : 2 * dp + 2, :],
                    )

                for dp in range(DT // 2):
                    pending.append(lambda dp=dp: unit(dp))
                pending.append(lambda: nc.sync.dma_start(w1s[g], wt[:]))
                return wt

            def sched_fold_w2(i):
                """Queue fold of W2' row-tile i as 2 half units."""
                raw = w2raw.tile([128, D], BF16, tag="w2raw")
                nc.gpsimd.dma_start(raw[:], w2r[:, i, :])

                def unit(ds):
                    dsl = slice(ds * TS, (ds + 1) * TS)
                    pw = ps.tile([128, TS], F32, tag="pf", bufs=4, name="pwf")
                    nc.tensor.matmul(
                        pw[:],
                        a2t_sb[:, i * 128 : (i + 1) * 128],
                        b2_sb[:, dsl],
                        start=True, stop=True,
                    )
                    nc.vector.tensor_add(w2sb[:, i, dsl], pw[:], raw[:, dsl])

                for ds in range(2):
                    pending.append(lambda ds=ds: unit(ds))

            def load_w1_group(g):
                wt = w1c.tile([128, DT, 256], BF16, tag="w1c")
                nc.sync.dma_start(wt[:], w1s[g])
                return wt

            NPAIR = FFT // 2  # 11 group pairs
            for blk in range(T_CORE // BLK):
                t0 = blk * BLK
                xt = xp.tile([128, DT, BLK], BF16, tag="xt")
                nc.scalar.dma_start(xt[:], xTr[:, :, t0 : t0 + BLK])
                h = hp.tile([128, FFT, BLK], BF16, tag="h")
                gtiles = {}
                # prologue: make group pair 0 available before the f-loop
                if blk == 0:
                    gtiles[0] = sched_fold_w1(0)
                    gtiles[NPAIR] = sched_fold_w1(NPAIR)
                    drain(99)
                else:
                    gtiles[0] = load_w1_group(0)
                    gtiles[NPAIR] = load_w1_group(NPAIR)
                # ---- phase 1: h = silu(x@W1g') * (x@W1u') ----
                for f in range(FFT):
                    g_gate, g_up = f // 2, NPAIR + f // 2
                    if f % 2 == 0 and f // 2 + 1 < NPAIR:
                        # stage next group pair one pair ahead of use
                        if blk == 0:
                            gtiles[g_gate + 1] = sched_fold_w1(g_gate + 1)
                            gtiles[g_up + 1] = sched_fold_w1(g_up + 1)
                        else:
                            gtiles[g_gate + 1] = load_w1_group(g_gate + 1)
                            gtiles[g_up + 1] = load_w1_group(g_up + 1)
                    if blk == 0:
                        sched_fold_w2(f)
                    off = (f % 2) * 128
                    gt, ut = gtiles[g_gate], gtiles[g_up]
                    pg0 = ps.tile([128, TS], F32, tag="pg", bufs=2, name="pg0")
                    pg1 = ps.tile([128, TS], F32, tag="pg", bufs=2, name="pg1")
                    for dt in range(DT):
                        lw = gt[:, dt, off : off + 128]
                        nc.tensor.matmul(pg0[:], lw, xt[:, dt, 0:TS],
                                         start=(dt == 0), stop=(dt == DT - 1))
                        nc.tensor.matmul(pg1[:], lw, xt[:, dt, TS:BLK],
                                         start=(dt == 0), stop=(dt == DT - 1))
                    tmp0 = tmpp.tile([128, TS], BF16, tag="tmp")
                    nc.scalar.activation(tmp0[:], pg0[:], SILU)
                    tmp1 = tmpp.tile([128, TS], BF16, tag="tmp")
                    nc.scalar.activation(tmp1[:], pg1[:], SILU)
                    drain(4)
                    pu0 = ps.tile([128, TS], F32, tag="pu", bufs=2, name="pu0")
                    pu1 = ps.tile([128, TS], F32, tag="pu", bufs=2, name="pu1")
                    for dt in range(DT):
                        lw = ut[:, dt, off : off + 128]
                        nc.tensor.matmul(pu0[:], lw, xt[:, dt, 0:TS],
                                         start=(dt == 0), stop=(dt == DT - 1))
                        nc.tensor.matmul(pu1[:], lw, xt[:, dt, TS:BLK],
                                         start=(dt == 0), stop=(dt == DT - 1))
                    nc.vector.tensor_mul(h[:, f, 0:TS], tmp0[:], pu0[:])
                    nc.vector.tensor_mul(h[:, f, TS:BLK], tmp1[:], pu1[:])
                    drain(4)
                drain(99)
                # ---- phase 2: out = h.T @ W2' ----
                for tt in range(BLK // 128):
                    ttl = slice(tt * 128, (tt + 1) * 128)
                    po0 = ps.tile([128, TS], F32, tag="pf", bufs=4, name="po0")
                    po1 = ps.tile([128, TS], F32, tag="pf", bufs=4, name="po1")
                    for i in range(FFT):
                        lw = h[:, i, ttl]
                        nc.tensor.matmul(po0[:], lw, w2sb[:, i, 0:TS],
                                         start=(i == 0), stop=(i == FFT - 1))
                        nc.tensor.matmul(po1[:], lw, w2sb[:, i, TS:D],
                                         start=(i == 0), stop=(i == FFT - 1))
                    ev = evp.tile([128, D], F32, tag="ev")
                    nc.vector.tensor_copy(ev[:, 0:TS], po0[:])
                    nc.scalar.activation(ev[:, TS:D], po1[:], COPY)
                    nc.scalar.dma_start(out[t0 + tt * 128 : t0 + (tt + 1) * 128, :], ev[:])
    nc.compile()
    return nc


def _get_prog():
    if "nc" not in _prog_cache:
        _prog_cache["nc"] = _build()
    return _prog_cache["nc"]


def run_sharded(inputs, trace=False, tmpdir=None):
    nc = _get_prog()
    x = inputs["x"]
    bf = lambda a: np.ascontiguousarray(a, dtype=BF)
    weights = {
        "W_gate_up": bf(inputs["W_gate_up"]),
        "B_gate_up": bf(inputs["B_gate_up"]),
        "A1T": bf(np.asarray(inputs["A_gate_up"]).T),
        "W_down": bf(inputs["W_down"]),
        "A2T": bf(np.asarray(inputs["A_down"]).T),
        "B_down": bf(inputs["B_down"]),
    }
    in_maps = []
    for c in range(N_CORES):
        xs = bf(np.asarray(x[c * T_CORE : (c + 1) * T_CORE]).T)
        in_maps.append({"xT": xs, **weights})
    res = run_bass_kernel_spmd(
        nc, in_maps, list(range(N_CORES)), trace=trace, tmpdir=tmpdir
    )
    outs = [res.results[c]["out"] for c in range(N_CORES)]
    full = np.concatenate(outs, axis=0)
    return full, res


def kernel(**inputs):
    full, _ = run_sharded(inputs, trace=False)
    return full


# revision 11
# speedup vs baseline: 1.1869x; 1.1869x over previous
"""LoRA MLP (gate_up + SiLU*up + down, each with rank-16 LoRA) on 8 TRN2 cores.

Strategy: data-parallel over tokens (16384 = 8 x 2048); weights replicated to
every core, no collectives. All tensors bf16 (PE full rate + FWL fast weight
load + half DMA/SBUF), fp32 PSUM accumulation, fp32 output.

The rank-16 LoRA is folded into the base weights on device:
    W1' = W_gate_up + A_gate_up @ B_gate_up   (PE matmul K=16 + DVE add)
    W2' = W_down    + A_down    @ B_down
so the steady-state loop is a pure dense MLP: clean 8-deep / 22-deep PSUM
accumulation chains with no 16-row LoRA matmuls serializing the PE.

Because the PE instruction queue is strict FIFO, fold work is emitted one
group-pair AHEAD of its consumer and chopped into small units (2 matmuls +
1 DVE add) that are interleaved between the main matmul chunks — by the time
the PE reaches a fold matmul its PSUM slot is long free, so the PE never
stalls on the fold's DVE evacuation chain (which caused HAM re-throttling).

Per core: 2 blocks of 1024 tokens. W2' (44KB/partition) stays SBUF-resident;
W1' groups are folded during block 0 and round-tripped through a DRAM scratch
for block 1 (write and read share one FIFO DMA queue, which orders them).
Activations stay in [feature, token] layout; each LDWEIGHTS feeds 2 matmuls.
DMA spread: weights on sync, x on vector, consts+W2raw on gpsimd, out on
scalar. PSUM: gate 2 + up 2 + shared fold/down-proj 4 banks.
"""

from collections import deque

import numpy as np
import ml_dtypes

import concourse.mybir as mybir
import concourse.tile as tile
from concourse import bacc
from concourse.bass_utils import run_bass_kernel_spmd

TOKENS, D, FF, R = 16384, 1024, 2816, 16
N_CORES = 8
T_CORE = TOKENS // N_CORES  # 2048
BLK = 1024                  # tokens per block (2 blocks/core)
TS = 512                    # psum free-dim tile (1 bank fp32)
DT = D // 128               # 8 d-model tiles
FFT = FF // 128             # 22 ff tiles
NG = 2 * FF // 256          # 22 fold groups of 256 f-cols each
F32 = mybir.dt.float32
BF16 = mybir.dt.bfloat16
SILU = mybir.ActivationFunctionType.Silu
COPY = mybir.ActivationFunctionType.Copy
BF = ml_dtypes.bfloat16

_prog_cache = {}


def _build():
    nc = bacc.Bacc("TRN2", target_bir_lowering=False, debug=False)
    xT = nc.dram_tensor("xT", [D, T_CORE], BF16, kind="ExternalInput").ap()
    w1 = nc.dram_tensor("W_gate_up", [D, 2 * FF], BF16, kind="ExternalInput").ap()
    a1t = nc.dram_tensor("A1T", [R, D], BF16, kind="ExternalInput").ap()
    b1 = nc.dram_tensor("B_gate_up", [R, 2 * FF], BF16, kind="ExternalInput").ap()
    w2 = nc.dram_tensor("W_down", [FF, D], BF16, kind="ExternalInput").ap()
    a2t = nc.dram_tensor("A2T", [R, FF], BF16, kind="ExternalInput").ap()
    b2 = nc.dram_tensor("B_down", [R, D], BF16, kind="ExternalInput").ap()
    out = nc.dram_tensor("out", [T_CORE, D], F32, kind="ExternalOutput").ap()
    # W1' spill space so block 1 re-reads the folded weights instead of refolding
    w1s = nc.dram_tensor("w1s", [NG, 128, DT, 256], BF16, kind="Internal").ap()

    w1r = w1.rearrange("(dt p) f -> p dt f", p=128)   # [128, 8, 5632]
    w2r = w2.rearrange("(ft p) d -> p ft d", p=128)   # [128, 22, 1024]
    xTr = xT.rearrange("(dt p) t -> p dt t", p=128)   # [128, 8, 2048]

    with tile.TileContext(nc) as tc:
        with (
            tc.tile_pool(name="constp", bufs=1) as constp,
            tc.tile_pool(name="w1c", bufs=7) as w1c,      # W1' group tiles, 4KB each
            tc.tile_pool(name="w2p", bufs=1) as w2p,      # W2' resident
            tc.tile_pool(name="w1raw", bufs=3) as w1raw,
            tc.tile_pool(name="w2raw", bufs=2) as w2raw,
            tc.tile_pool(name="b1p", bufs=4) as b1p,
            tc.tile_pool(name="xp", bufs=2) as xp,
            tc.tile_pool(name="hp", bufs=1) as hp,
            tc.tile_pool(name="tmpp", bufs=4) as tmpp,
            tc.tile_pool(name="evp", bufs=2) as evp,
            tc.tile_pool(name="ps", bufs=1, space="PSUM") as ps,
        ):
            a1t_sb = constp.tile([R, D], BF16)
            nc.gpsimd.dma_start(a1t_sb[:], a1t[:])
            a2t_sb = constp.tile([R, FF], BF16)
            nc.gpsimd.dma_start(a2t_sb[:], a2t[:])
            b2_sb = constp.tile([R, D], BF16)
            nc.gpsimd.dma_start(b2_sb[:], b2[:])
            w2sb = w2p.tile([128, FFT, D], BF16)

            pending = deque()  # fold micro-units, drained between main MM chunks

            def drain(n):
                for _ in range(min(n, len(pending))):
                    pending.popleft()()

            def sched_fold_w1(g):
                """Queue fold of W1' cols [g*256,(g+1)*256): 4 units + spill."""
                c0 = g * 256
                raw = w1raw.tile([128, DT, 256], BF16, tag="w1raw")
                nc.sync.dma_start(raw[:], w1r[:, :, c0 : c0 + 256])
                b1c = b1p.tile([R, 256], BF16, tag="b1c")
                nc.sync.dma_start(b1c[:], b1[:, c0 : c0 + 256])
                wt = w1c.tile([128, DT, 256], BF16, tag="w1c")

                def unit(dp):
                    pf = ps.tile([128, 2, 256], F32, tag="pf", bufs=4, name="pf")
                    for k in range(2):
                        dt = 2 * dp + k
                        nc.tensor.matmul(
                            pf[:, k, :],
                            a1t_sb[:, dt * 128 : (dt + 1) * 128],
                            b1c[:],
                            start=True, stop=True,
                        )
                    nc.vector.tensor_add(
                        wt[:, 2 * dp : 2 * dp + 2, :], pf[:],
                        raw[:, 2 * dp : 2 * dp + 2, :],
                    )

                for dp in range(DT // 2):
                    pending.append(lambda dp=dp: unit(dp))
                pending.append(lambda: nc.gpsimd.dma_start(w1s[g], wt[:]))
                return wt

            def sched_fold_w2(i):
                """Queue fold of W2' row-tile i as 2 half units."""
                raw = w2raw.tile([128, D], BF16, tag="w2raw")
                nc.gpsimd.dma_start(raw[:], w2r[:, i, :])

                def unit(ds):
                    dsl = slice(ds * TS, (ds + 1) * TS)
                    pw = ps.tile([128, TS], F32, tag="pf", bufs=4, name="pwf")
                    nc.tensor.matmul(
                        pw[:],
                        a2t_sb[:, i * 128 : (i + 1) * 128],
                        b2_sb[:, dsl],
                        start=True, stop=True,
                    )
                    nc.vector.tensor_add(w2sb[:, i, dsl], pw[:], raw[:, dsl])

                for ds in range(2):
                    pending.append(lambda ds=ds: unit(ds))

            def load_w1_group(g):
                wt = w1c.tile([128, DT, 256], BF16, tag="w1c")
                nc.gpsimd.dma_start(wt[:], w1s[g])
                return wt

            NPAIR = FFT // 2  # 11 group pairs
            for blk in range(T_CORE // BLK):
                t0 = blk * BLK
                xt = xp.tile([128, DT, BLK], BF16, tag="xt")
                nc.scalar.dma_start(xt[:, 0 : DT // 2, :], xTr[:, 0 : DT // 2, t0 : t0 + BLK])
                nc.gpsimd.dma_start(xt[:, DT // 2 : DT, :], xTr[:, DT // 2 : DT, t0 : t0 + BLK])
                h = hp.tile([128, FFT, BLK], BF16, tag="h")
                gtiles = {}
                # prologue: make group pair 0 available before the f-loop
                if blk == 0:
                    gtiles[0] = sched_fold_w1(0)
                    gtiles[NPAIR] = sched_fold_w1(NPAIR)
                    drain(99)
                else:
                    gtiles[0] = load_w1_group(0)
                    gtiles[NPAIR] = load_w1_group(NPAIR)
                # ---- phase 1: h = silu(x@W1g') * (x@W1u') ----
                for f in range(FFT):
                    g_gate, g_up = f // 2, NPAIR + f // 2
                    if f % 2 == 0 and f // 2 + 1 < NPAIR:
                        # stage next group pair one pair ahead of use
                        if blk == 0:
                            gtiles[g_gate + 1] = sched_fold_w1(g_gate + 1)
                            gtiles[g_up + 1] = sched_fold_w1(g_up + 1)
                        else:
                            gtiles[g_gate + 1] = load_w1_group(g_gate + 1)
                            gtiles[g_up + 1] = load_w1_group(g_up + 1)
                    if blk == 0:
                        sched_fold_w2(f)
                    off = (f % 2) * 128
                    gt, ut = gtiles[g_gate], gtiles[g_up]
                    pg0 = ps.tile([128, TS], F32, tag="pg", bufs=2, name="pg0")
                    pg1 = ps.tile([128, TS], F32, tag="pg", bufs=2, name="pg1")
                    for dt in range(DT):
                        lw = gt[:, dt, off : off + 128]
                        nc.tensor.matmul(pg0[:], lw, xt[:, dt, 0:TS],
                                         start=(dt == 0), stop=(dt == DT - 1))
                        nc.tensor.matmul(pg1[:], lw, xt[:, dt, TS:BLK],
                                         start=(dt == 0), stop=(dt == DT - 1))
                    tmp0 = tmpp.tile([128, TS], BF16, tag="tmp")
                    nc.scalar.activation(tmp0[:], pg0[:], SILU)
                    tmp1 = tmpp.tile([128, TS], BF16, tag="tmp")
                    nc.scalar.activation(tmp1[:], pg1[:], SILU)
                    drain(4)
                    pu0 = ps.tile([128, TS], F32, tag="pu", bufs=2, name="pu0")
                    pu1 = ps.tile([128, TS], F32, tag="pu", bufs=2, name="pu1")
                    for dt in range(DT):
                        lw = ut[:, dt, off : off + 128]
                        nc.tensor.matmul(pu0[:], lw, xt[:, dt, 0:TS],
                                         start=(dt == 0), stop=(dt == DT - 1))
                        nc.tensor.matmul(pu1[:], lw, xt[:, dt, TS:BLK],
                                         start=(dt == 0), stop=(dt == DT - 1))
                    nc.vector.tensor_mul(h[:, f, 0:TS], tmp0[:], pu0[:])
                    nc.vector.tensor_mul(h[:, f, TS:BLK], tmp1[:], pu1[:])
                    drain(4)
                drain(99)
                # ---- phase 2: out = h.T @ W2' ----
                for tt in range(BLK // 128):
                    ttl = slice(tt * 128, (tt + 1) * 128)
                    po0 = ps.tile([128, TS], F32, tag="pf", bufs=4, name="po0")
                    po1 = ps.tile([128, TS], F32, tag="pf", bufs=4, name="po1")
                    for i in range(FFT):
                        lw = h[:, i, ttl]
                        nc.tensor.matmul(po0[:], lw, w2sb[:, i, 0:TS],
                                         start=(i == 0), stop=(i == FFT - 1))
                        nc.tensor.matmul(po1[:], lw, w2sb[:, i, TS:D],
                                         start=(i == 0), stop=(i == FFT - 1))
                    ev = evp.tile([128, D], F32, tag="ev")
                    nc.vector.tensor_copy(ev[:, 0:TS], po0[:])
                    nc.scalar.activation(ev[:, TS:D], po1[:], COPY)
                    nc.scalar.dma_start(out[t0 + tt * 128 : t0 + (tt + 1) * 128, :], ev[:])
    nc.compile()
    return nc


def _get_prog():
    if "nc" not in _prog_cache:
        _prog_cache["nc"] = _build()
    return _prog_cache["nc"]


def run_sharded(inputs, trace=False, tmpdir=None):
    nc = _get_prog()
    x = inputs["x"]
    bf = lambda a: np.ascontiguousarray(a, dtype=BF)
    weights = {
        "W_gate_up": bf(inputs["W_gate_up"]),
        "B_gate_up": bf(inputs["B_gate_up"]),
        "A1T": bf(np.asarray(inputs["A_gate_up"]).T),
        "W_down": bf(inputs["W_down"]),
        "A2T": bf(np.asarray(inputs["A_down"]).T),
        "B_down": bf(inputs["B_down"]),
    }
    in_maps = []
    for c in range(N_CORES):
        xs = bf(np.asarray(x[c * T_CORE : (c + 1) * T_CORE]).T)
        in_maps.append({"xT": xs, **weights})
    res = run_bass_kernel_spmd(
        nc, in_maps, list(range(N_CORES)), trace=trace, tmpdir=tmpdir
    )
    outs = [res.results[c]["out"] for c in range(N_CORES)]
    full = np.concatenate(outs, axis=0)
    return full, res


def kernel(**inputs):
    full, _ = run_sharded(inputs, trace=False)
    return full


# revision 15
# speedup vs baseline: 1.2042x; 1.0146x over previous
"""LoRA MLP (gate_up + SiLU*up + down, each with rank-16 LoRA) on 8 TRN2 cores.

Strategy: data-parallel over tokens (16384 = 8 x 2048); weights replicated to
every core, no collectives. All tensors bf16 (PE full rate + FWL fast weight
load + half DMA/SBUF), fp32 PSUM accumulation, fp32 output.

The rank-16 LoRA is folded into the base weights on device:
    W1' = W_gate_up + A_gate_up @ B_gate_up   (PE matmul K=16 + DVE add)
    W2' = W_down    + A_down    @ B_down
so the steady-state loop is a pure dense MLP: clean 8-deep / 22-deep PSUM
accumulation chains with no 16-row LoRA matmuls serializing the PE.

Because the PE instruction queue is strict FIFO, fold work is emitted one
group-pair AHEAD of its consumer and chopped into small units (2 matmuls +
1 DVE add) that are interleaved between the main matmul chunks — by the time
the PE reaches a fold matmul its PSUM slot is long free, so the PE never
stalls on the fold's DVE evacuation chain (which caused HAM re-throttling).

Per core: 2 blocks of 1024 tokens. W2' (44KB/partition) stays SBUF-resident;
W1' groups are folded during block 0 and round-tripped through a DRAM scratch
for block 1 (write and read share one FIFO DMA queue, which orders them).
Activations stay in [feature, token] layout; each LDWEIGHTS feeds 2 matmuls.
DMA spread: weights on sync, x on vector, consts+W2raw on gpsimd, out on
scalar. PSUM: gate 2 + up 2 + shared fold/down-proj 4 banks.
"""

from collections import deque

import numpy as np
import ml_dtypes

import concourse.mybir as mybir
import concourse.tile as tile
from concourse import bacc
from concourse.bass_utils import run_bass_kernel_spmd

TOKENS, D, FF, R = 16384, 1024, 2816, 16
N_CORES = 8
T_CORE = TOKENS // N_CORES  # 2048
BLK = 1024                  # tokens per block (2 blocks/core)
TS = 512                    # psum free-dim tile (1 bank fp32)
DT = D // 128               # 8 d-model tiles
FFT = FF // 128             # 22 ff tiles
NG = 2 * FF // 256          # 22 fold groups of 256 f-cols each
F32 = mybir.dt.float32
BF16 = mybir.dt.bfloat16
SILU = mybir.ActivationFunctionType.Silu
COPY = mybir.ActivationFunctionType.Copy
BF = ml_dtypes.bfloat16

_prog_cache = {}


def _build():
    nc = bacc.Bacc("TRN2", target_bir_lowering=False, debug=False)
    xT = nc.dram_tensor("xT", [D, T_CORE], BF16, kind="ExternalInput").ap()
    # W1G/W2G are host-pre-grouped so every weight DMA is contiguous per partition
    w1g = nc.dram_tensor("W1G", [NG, 128, DT, 256], BF16, kind="ExternalInput").ap()
    a1t = nc.dram_tensor("A1T", [R, D], BF16, kind="ExternalInput").ap()
    b1 = nc.dram_tensor("B_gate_up", [R, 2 * FF], BF16, kind="ExternalInput").ap()
    w2g = nc.dram_tensor("W2G", [FFT, 128, D], BF16, kind="ExternalInput").ap()
    a2t = nc.dram_tensor("A2T", [R, FF], BF16, kind="ExternalInput").ap()
    b2 = nc.dram_tensor("B_down", [R, D], BF16, kind="ExternalInput").ap()
    out = nc.dram_tensor("out", [T_CORE, D], F32, kind="ExternalOutput").ap()
    # W1' spill space so block 1 re-reads the folded weights instead of refolding
    w1s = nc.dram_tensor("w1s", [NG, 128, DT, 256], BF16, kind="Internal").ap()

    xTr = xT.rearrange("(dt p) t -> p dt t", p=128)   # [128, 8, 2048]

    with tile.TileContext(nc) as tc:
        with (
            tc.tile_pool(name="constp", bufs=1) as constp,
            tc.tile_pool(name="w1c", bufs=7) as w1c,      # W1' group tiles, 4KB each
            tc.tile_pool(name="w2p", bufs=1) as w2p,      # W2' resident
            tc.tile_pool(name="w1raw", bufs=3) as w1raw,
            tc.tile_pool(name="w2raw", bufs=2) as w2raw,
            tc.tile_pool(name="b1p", bufs=4) as b1p,
            tc.tile_pool(name="xp", bufs=2) as xp,
            tc.tile_pool(name="hp", bufs=1) as hp,
            tc.tile_pool(name="tmpp", bufs=4) as tmpp,
            tc.tile_pool(name="evp", bufs=2) as evp,
            tc.tile_pool(name="ps", bufs=1, space="PSUM") as ps,
        ):
            a1t_sb = constp.tile([R, D], BF16)
            nc.gpsimd.dma_start(a1t_sb[:], a1t[:])
            a2t_sb = constp.tile([R, FF], BF16)
            nc.gpsimd.dma_start(a2t_sb[:], a2t[:])
            b2_sb = constp.tile([R, D], BF16)
            nc.gpsimd.dma_start(b2_sb[:], b2[:])
            w2sb = w2p.tile([128, FFT, D], BF16)

            pending = deque()  # fold micro-units, drained between main MM chunks

            def drain(n):
                for _ in range(min(n, len(pending))):
                    pending.popleft()()

            def sched_fold_w1(g):
                """Queue fold of W1' cols [g*256,(g+1)*256): 4 units + spill."""
                c0 = g * 256
                raw = w1raw.tile([128, DT, 256], BF16, tag="w1raw")
                nc.sync.dma_start(raw[:], w1g[g])
                b1c = b1p.tile([R, 256], BF16, tag="b1c")
                nc.sync.dma_start(b1c[:], b1[:, c0 : c0 + 256])
                wt = w1c.tile([128, DT, 256], BF16, tag="w1c")

                def unit(dp):
                    pf = ps.tile([128, 2, 256], F32, tag="pf", bufs=4, name="pf")
                    for k in range(2):
                        dt = 2 * dp + k
                        nc.tensor.matmul(
                            pf[:, k, :],
                            a1t_sb[:, dt * 128 : (dt + 1) * 128],
                            b1c[:],
                            start=True, stop=True,
                        )
                    nc.vector.tensor_add(
                        wt[:, 2 * dp : 2 * dp + 2, :], pf[:],
                        raw[:, 2 * dp : 2 * dp + 2, :],
                    )

                for dp in range(DT // 2):
                    pending.append(lambda dp=dp: unit(dp))
                pending.append(lambda: nc.gpsimd.dma_start(w1s[g], wt[:]))
                return wt

            def sched_fold_w2(i):
                """Queue fold of W2' row-tile i as 2 half units."""
                raw = w2raw.tile([128, D], BF16, tag="w2raw")
                nc.gpsimd.dma_start(raw[:], w2g[i])

                def unit(ds):
                    dsl = slice(ds * TS, (ds + 1) * TS)
                    pw = ps.tile([128, TS], F32, tag="pf", bufs=4, name="pwf")
                    nc.tensor.matmul(
                        pw[:],
                        a2t_sb[:, i * 128 : (i + 1) * 128],
                        b2_sb[:, dsl],
                        start=True, stop=True,
                    )
                    nc.vector.tensor_add(w2sb[:, i, dsl], pw[:], raw[:, dsl])

                for ds in range(2):
                    pending.append(lambda ds=ds: unit(ds))

            def load_w1_group(g):
                wt = w1c.tile([128, DT, 256], BF16, tag="w1c")
                nc.gpsimd.dma_start(wt[:], w1s[g])
                return wt

            NPAIR = FFT // 2  # 11 group pairs
            for blk in range(T_CORE // BLK):
                t0 = blk * BLK
                xt = xp.tile([128, DT, BLK], BF16, tag="xt")
                nc.scalar.dma_start(xt[:, 0 : DT // 2, :], xTr[:, 0 : DT // 2, t0 : t0 + BLK])
                nc.gpsimd.dma_start(xt[:, DT // 2 : DT, :], xTr[:, DT // 2 : DT, t0 : t0 + BLK])
                h = hp.tile([128, FFT, BLK], BF16, tag="h")
                gtiles = {}
                # prologue: make group pair 0 available before the f-loop
                if blk == 0:
                    gtiles[0] = sched_fold_w1(0)
                    gtiles[NPAIR] = sched_fold_w1(NPAIR)
                    drain(99)
                else:
                    gtiles[0] = load_w1_group(0)
                    gtiles[NPAIR] = load_w1_group(NPAIR)
                # ---- phase 1: h = silu(x@W1g') * (x@W1u') ----
                for f in range(FFT):
                    g_gate, g_up = f // 2, NPAIR + f // 2
                    if f % 2 == 0 and f // 2 + 1 < NPAIR:
                        # stage next group pair one pair ahead of use
                        if blk == 0:
                            gtiles[g_gate + 1] = sched_fold_w1(g_gate + 1)
                            gtiles[g_up + 1] = sched_fold_w1(g_up + 1)
                        else:
                            gtiles[g_gate + 1] = load_w1_group(g_gate + 1)
                            gtiles[g_up + 1] = load_w1_group(g_up + 1)
                    if blk == 0:
                        sched_fold_w2(f)
                    off = (f % 2) * 128
                    gt, ut = gtiles[g_gate], gtiles[g_up]
                    pg0 = ps.tile([128, TS], F32, tag="pg", bufs=2, name="pg0")
                    pg1 = ps.tile([128, TS], F32, tag="pg", bufs=2, name="pg1")
                    for dt in range(DT):
                        lw = gt[:, dt, off : off + 128]
                        nc.tensor.matmul(pg0[:], lw, xt[:, dt, 0:TS],
                                         start=(dt == 0), stop=(dt == DT - 1))
                        nc.tensor.matmul(pg1[:], lw, xt[:, dt, TS:BLK],
                                         start=(dt == 0), stop=(dt == DT - 1))
                    tmp0 = tmpp.tile([128, TS], BF16, tag="tmp")
                    nc.scalar.activation(tmp0[:], pg0[:], SILU)
                    tmp1 = tmpp.tile([128, TS], BF16, tag="tmp")
                    nc.scalar.activation(tmp1[:], pg1[:], SILU)
                    drain(4)
                    pu0 = ps.tile([128, TS], F32, tag="pu", bufs=2, name="pu0")
                    pu1 = ps.tile([128, TS], F32, tag="pu", bufs=2, name="pu1")
                    for dt in range(DT):
                        lw = ut[:, dt, off : off + 128]
                        nc.tensor.matmul(pu0[:], lw, xt[:, dt, 0:TS],
                                         start=(dt == 0), stop=(dt == DT - 1))
                        nc.tensor.matmul(pu1[:], lw, xt[:, dt, TS:BLK],
                                         start=(dt == 0), stop=(dt == DT - 1))
                    nc.vector.tensor_mul(h[:, f, 0:TS], tmp0[:], pu0[:])
                    nc.vector.tensor_mul(h[:, f, TS:BLK], tmp1[:], pu1[:])
                    drain(4)
                drain(99)
                # ---- phase 2: out = h.T @ W2' ----
                for tt in range(BLK // 128):
                    ttl = slice(tt * 128, (tt + 1) * 128)
                    po0 = ps.tile([128, TS], F32, tag="pf", bufs=4, name="po0")
                    po1 = ps.tile([128, TS], F32, tag="pf", bufs=4, name="po1")
                    for i in range(FFT):
                        lw = h[:, i, ttl]
                        nc.tensor.matmul(po0[:], lw, w2sb[:, i, 0:TS],
                                         start=(i == 0), stop=(i == FFT - 1))
                        nc.tensor.matmul(po1[:], lw, w2sb[:, i, TS:D],
                                         start=(i == 0), stop=(i == FFT - 1))
                    ev = evp.tile([128, D], F32, tag="ev")
                    nc.vector.tensor_copy(ev[:, 0:TS], po0[:])
                    nc.scalar.activation(ev[:, TS:D], po1[:], COPY)
                    nc.scalar.dma_start(out[t0 + tt * 128 : t0 + (tt + 1) * 128, :], ev[:])
    nc.compile()
    return nc


def _get_prog():
    if "nc" not in _prog_cache:
        _prog_cache["nc"] = _build()
    return _prog_cache["nc"]


def run_sharded(inputs, trace=False, tmpdir=None):
    nc = _get_prog()
    x = inputs["x"]
    bf = lambda a: np.ascontiguousarray(a, dtype=BF)
    # group W1 as [22 groups, 128 partitions, 8 d-tiles, 256 f-cols], W2 as
    # [22 ff-tiles, 128 partitions, 1024] so kernel weight DMAs are contiguous
    w1grp = (
        np.asarray(inputs["W_gate_up"])
        .reshape(DT, 128, NG, 256)
        .transpose(2, 1, 0, 3)
    )
    w2grp = np.asarray(inputs["W_down"]).reshape(FFT, 128, D)
    weights = {
        "W1G": bf(w1grp),
        "B_gate_up": bf(inputs["B_gate_up"]),
        "A1T": bf(np.asarray(inputs["A_gate_up"]).T),
        "W2G": bf(w2grp),
        "A2T": bf(np.asarray(inputs["A_down"]).T),
        "B_down": bf(inputs["B_down"]),
    }
    in_maps = []
    for c in range(N_CORES):
        xs = bf(np.asarray(x[c * T_CORE : (c + 1) * T_CORE]).T)
        in_maps.append({"xT": xs, **weights})
    res = run_bass_kernel_spmd(
        nc, in_maps, list(range(N_CORES)), trace=trace, tmpdir=tmpdir
    )
    outs = [res.results[c]["out"] for c in range(N_CORES)]
    full = np.concatenate(outs, axis=0)
    return full, res


def kernel(**inputs):
    full, _ = run_sharded(inputs, trace=False)
    return full
